# revision 12
# baseline (speedup 1.0000x reference)
"""Trainium2 Bass kernel for LoRALinear: out = x @ W^T + bias + scaling * (x @ A^T) @ B^T.

Problem shapes (hardcoded): x [4, 2048, 4096] f32, weight [4096, 4096] f32,
bias [4096] f32, lora_A [16, 4096] f32, lora_B [4096, 16] f32, scaling = 2.0.

Strategy: pure data-parallel over the 8192 token rows across 8 NeuronCores
(1024 rows each, no collectives). Host-side prep folds the LoRA update into
the weight (W_eff = W + scaling * B @ A — exact in fp32), transposes and
packs operands so the contraction dim lands on SBUF partitions, and
pre-broadcasts bias to 128 partitions so the bias add rides the PSUM->SBUF
copy on the vector engine instead of costing PE matmuls.

Precision split (mode='hybrid'): the first 1024 of the 4096 contraction runs
in fp8-e4m3 with DoubleRow perf mode (2 contraction elements per PE cell per
cycle), the remaining 3072 in fp16. Both accumulate in separate fp32 PSUM
banks; the vector engine combines  out = ps8 * (1/64) + bias + ps16  (w8 is
pre-scaled by 64 on the host so its N(0, 1/64) entries clear e4m3's denormal
floor). Measured end-to-end rel err 1.6e-2 vs the fp32 reference (gate 2e-2);
pure-fp16 mode stays at 2.5e-4.

Per core: out[1024, 4096] = xT.T @ wT + bias with
  - x resident in SBUF as 8 per-row-tile chunks (fast pipeline start),
  - wT streamed in k-slab tiles so matmuls only wait on the slab they read,
  - PSUM double-buffered across row tiles (8 banks in hybrid mode).
"""

import json

import numpy as np

import concourse.mybir as mybir
import concourse.tile as tile
from concourse import bacc, bass_utils

N_CORES = 8
B, S, D_IN, D_OUT, R = 4, 2048, 4096, 4096, 16
SCALING = 2.0
M_TOTAL = B * S              # 8192
M_CORE = M_TOTAL // N_CORES  # 1024
P = 128
KO = D_IN // P               # 32 contraction tiles
N_SLICE = 512
SLAB = 8                     # contraction tiles per w slab
M_TILES = M_CORE // P        # 8
F16 = mybir.dt.float16
F8 = mybir.dt.float8e4
F32 = mybir.dt.float32

K8 = 1024                    # contraction length done in fp8 (hybrid mode)
Q8 = K8 // 256               # DoubleRow matmuls per n-slice (K=256 each)
KO16_H = (D_IN - K8) // P    # 24 fp16 contraction tiles in hybrid mode
W8_SCALE = 64.0

DEFAULT_MODE = 'hybrid'
DEFAULT_G = 2


def surgery_dedup_ldweights(nc, expect_removed=None):
    """Remove back-to-back duplicate InstLdweights (same weights AP /
    tile_position / perf_mode) from the compiled module, merging any
    semaphore waits/updates into the next PE instruction. Verified
    bit-identical on hardware; measured perf-neutral for fp16 (the PE
    overlaps LdWeights with matmuls), kept as an option for experiments."""
    d = json.loads(mybir.module_to_json_string(nc.m))
    removed = skipped = 0
    for fn in d['functions']:
        for blk in fn['blocks']:
            insts = blk['instructions']
            out = []
            last_key = None
            pending = None
            for inst in insts:
                op = inst.get('opcode')
                eng = inst.get('engine')
                if op == 'Ldweights':
                    key = json.dumps(
                        [inst['ins'], inst.get('tile_position'),
                         inst.get('tile_size'), inst.get('perf_mode'),
                         inst.get('is_transpose')], sort_keys=True)
                    if key == last_key:
                        si = inst.get('sync_info') or {}
                        cand = {'on_wait': list(si.get('on_wait', [])),
                                'on_update': list(si.get('on_update', []))}
                        if pending:
                            cand['on_wait'] += pending['on_wait']
                            cand['on_update'] += pending['on_update']
                        if len(cand['on_wait']) <= 1 and len(cand['on_update']) <= 1:
                            pending = cand
                            removed += 1
                            continue
                        skipped += 1
                        last_key = key
                    else:
                        last_key = key
                elif op in ('Matmult', 'EventSemaphore'):
                    pass
                elif eng == 'PE':
                    last_key = None
                if pending is not None and eng == 'PE':
                    si = inst.setdefault('sync_info',
                                         {'on_update': [], 'on_wait': []})
                    cur = si.setdefault('on_wait', [])
                    for w in pending['on_wait']:
                        hit = False
                        for cw in cur:
                            if (cw.get('id') == w.get('id')
                                    and cw.get('sync_type') == w.get('sync_type')
                                    and cw.get('wait_mode') == w.get('wait_mode')):
                                cw['wait_value'] = max(cw['wait_value'],
                                                       w['wait_value'])
                                hit = True
                                break
                        if not hit:
                            cur.append(w)
                    si.setdefault('on_update', []).extend(pending['on_update'])
                    pending = None
                out.append(inst)
            assert pending is None, "dangling sync_info from removed Ldweights"
            blk['instructions'] = out
    if expect_removed is not None:
        assert removed >= 0.9 * expect_removed, (removed, skipped, expect_removed)
    nc.m = mybir.module_from_json_string(json.dumps(d))
    return removed


def build_nc(reps: int = 1, mode: str = DEFAULT_MODE, surgery: bool = False,
             g: int = DEFAULT_G):
    """Build and compile the per-core Bass program. reps>1 wraps the whole
    body in a hardware For_i loop (used only for timing runs). `g` = number
    of 512-wide n-slices computed together per k step."""
    hybrid = (mode == 'hybrid')
    ko16 = KO16_H if hybrid else KO
    nb_count = D_OUT // (g * N_SLICE)
    n_slabs = ko16 // SLAB
    nc = bacc.Bacc("TRN2", target_bir_lowering=False, debug=False,
                   num_devices=N_CORES)

    # x pre-swizzled on host: [mt, p, ko, m] so each chunk DMA is
    # partition-contiguous runs
    xT_d = nc.dram_tensor("xT", [M_TILES, P, ko16, P], F16,
                          kind="ExternalInput")
    wT_d = nc.dram_tensor("wT", [ko16 * P, D_OUT], F16, kind="ExternalInput")
    bias_d = nc.dram_tensor("bias", [P, D_OUT], F16, kind="ExternalInput")
    out_d = nc.dram_tensor("out", [M_CORE, D_OUT], F32, kind="ExternalOutput")
    if hybrid:
        x8_d = nc.dram_tensor("x8", [M_TILES, P, Q8, 2, P], F8,
                              kind="ExternalInput")
        w8_d = nc.dram_tensor("w8", [P, Q8, 2, D_OUT], F8,
                              kind="ExternalInput")

    xT_r = xT_d.ap()                                         # [8,128,ko16,128]
    wT_r = wT_d.ap().rearrange("(ko p) n -> p ko n", p=P)    # [128,ko16,4096]
    out_r = out_d.ap().rearrange("(mt p) n -> mt p n", p=P)  # [8, 128, 4096]

    w_bufs = 2 * n_slabs if g <= 2 else n_slabs

    with tile.TileContext(nc) as tc:
        with (
            tc.tile_pool(name="xp", bufs=M_TILES) as x_pool,
            tc.tile_pool(name="x8p", bufs=M_TILES) as x8_pool,
            tc.tile_pool(name="wp", bufs=w_bufs) as w_pool,
            tc.tile_pool(name="w8p", bufs=2) as w8_pool,
            tc.tile_pool(name="cst", bufs=2) as c_pool,
            tc.tile_pool(name="op", bufs=2) as o_pool,
            tc.tile_pool(name="ps", bufs=(2 if hybrid else 4),
                         space="PSUM") as ps_pool,
        ):
            def body(_i=None):
                x_tiles = []
                x8_tiles = []
                for mt in range(M_TILES):
                    xt = x_pool.tile([P, ko16, P], F16, name="xt")
                    nc.sync.dma_start(xt[:], xT_r[mt])
                    x_tiles.append(xt)
                    if hybrid:
                        x8t = x8_pool.tile([P, Q8, 2, P], F8, name="x8t")
                        nc.sync.dma_start(x8t[:], x8_d.ap()[mt])
                        x8_tiles.append(x8t)
                bias_sb = c_pool.tile([P, D_OUT], F16)
                nc.sync.dma_start(bias_sb[:], bias_d.ap())

                for nb in range(nb_count):
                    ncol0 = nb * g * N_SLICE
                    slabs = []
                    for s in range(n_slabs):
                        w_sb = w_pool.tile([P, SLAB, g * N_SLICE], F16,
                                           name="wsl")
                        nc.sync.dma_start(
                            w_sb[:],
                            wT_r[:, s * SLAB:(s + 1) * SLAB,
                                 ncol0:ncol0 + g * N_SLICE])
                        slabs.append(w_sb)
                    if hybrid:
                        w8_sb = w8_pool.tile([P, Q8, 2, g * N_SLICE], F8,
                                             name="w8")
                        nc.sync.dma_start(
                            w8_sb[:],
                            w8_d.ap()[:, :, :, ncol0:ncol0 + g * N_SLICE])
                    for mt in range(M_TILES):
                        pss = [ps_pool.tile([P, N_SLICE], F32, name=f"ps{j}")
                               for j in range(g)]
                        if hybrid:
                            ps8s = [ps_pool.tile([P, N_SLICE], F32,
                                                 name=f"ps8_{j}")
                                    for j in range(g)]
                            for q in range(Q8):
                                st = (q == 0)
                                sp = (q == Q8 - 1)
                                for j in range(g):
                                    nc.tensor.matmul(
                                        ps8s[j][:], x8_tiles[mt][:, q, :, :],
                                        w8_sb[:, q, :,
                                              j * N_SLICE:(j + 1) * N_SLICE],
                                        start=st, stop=sp,
                                        perf_mode=mybir.MatmulPerfMode.DoubleRow)
                        for k in range(ko16):
                            st = (k == 0)
                            sp = (k == ko16 - 1)
                            w_sb = slabs[k // SLAB]
                            ks = k % SLAB
                            for j in range(g):
                                nc.tensor.matmul(
                                    pss[j][:], x_tiles[mt][:, k, :],
                                    w_sb[:, ks, j * N_SLICE:(j + 1) * N_SLICE],
                                    start=st, stop=sp)
                        for j in range(g):
                            ncol = ncol0 + j * N_SLICE
                            if hybrid:
                                # o = ps8/64 + bias ; o2 = o + ps16
                                # (each DVE op reads a single PSUM operand)
                                o_sb = o_pool.tile([P, N_SLICE], F32,
                                                   name=f"o{j}")
                                nc.vector.scalar_tensor_tensor(
                                    o_sb[:], ps8s[j][:], 1.0 / W8_SCALE,
                                    bias_sb[:, ncol:ncol + N_SLICE],
                                    mybir.AluOpType.mult,
                                    mybir.AluOpType.add)
                                o2_sb = o_pool.tile([P, N_SLICE], F32,
                                                    name=f"o2_{j}")
                                nc.vector.tensor_add(
                                    o2_sb[:], pss[j][:], o_sb[:])
                                nc.sync.dma_start(
                                    out_r[mt, :, ncol:ncol + N_SLICE],
                                    o2_sb[:])
                            else:
                                o_sb = o_pool.tile([P, N_SLICE], F32,
                                                   name=f"o{j}")
                                nc.vector.tensor_add(
                                    o_sb[:], pss[j][:],
                                    bias_sb[:, ncol:ncol + N_SLICE])
                                nc.sync.dma_start(
                                    out_r[mt, :, ncol:ncol + N_SLICE],
                                    o_sb[:])

            if reps == 1:
                body()
            else:
                with tc.For_i(0, reps, 1) as i:
                    body(i)

    nc.compile()
    if surgery:
        surgery_dedup_ldweights(nc)
    return nc


_NC_CACHE = {}


def _get_nc(reps: int = 1, mode: str = DEFAULT_MODE, surgery: bool = False,
            g: int = DEFAULT_G):
    key = (reps, mode, surgery, g)
    if key not in _NC_CACHE:
        _NC_CACHE[key] = build_nc(reps, mode, surgery, g)
    return _NC_CACHE[key]


def prep_in_maps(x, weight, bias, lora_A, lora_B, mode: str = DEFAULT_MODE):
    """Host-side shard + pack: returns in_maps for the 8 cores."""
    import ml_dtypes
    E4 = ml_dtypes.float8_e4m3
    hybrid = (mode == 'hybrid')
    ko16 = KO16_H if hybrid else KO
    k16_lo = K8 if hybrid else 0

    xf = np.asarray(x, dtype=np.float32).reshape(M_TOTAL, D_IN)
    w_eff = np.asarray(weight, dtype=np.float32) + SCALING * (
        np.asarray(lora_B, dtype=np.float32) @ np.asarray(lora_A, dtype=np.float32))
    wT32 = np.ascontiguousarray(w_eff.T)                 # [K, N]
    wT = wT32[k16_lo:].astype(np.float16)
    bias_bc = np.ascontiguousarray(
        np.broadcast_to(np.asarray(bias, dtype=np.float32)
                        .astype(np.float16).reshape(1, D_OUT), (P, D_OUT)))
    if hybrid:
        # w8[p, q, j, n] = e4m3(64 * wT[k, n]), k = q*256 + j*128 + p
        w8 = np.ascontiguousarray(
            np.clip(wT32[:K8] * W8_SCALE, -240, 240)
            .reshape(Q8, 2, P, D_OUT).transpose(2, 0, 1, 3)).astype(E4)
    in_maps = []
    for c in range(N_CORES):
        xc = xf[c * M_CORE:(c + 1) * M_CORE]             # [1024, 4096] f32
        x16 = xc[:, k16_lo:].astype(np.float16)
        x_sw = np.ascontiguousarray(
            x16.reshape(M_TILES, P, ko16, P).transpose(0, 3, 2, 1))
        m = {"xT": x_sw, "wT": wT, "bias": bias_bc}
        if hybrid:
            # x8[mt, p, q, j, m] = e4m3(x[k, mt*128+m]), k = q*256 + j*128 + p
            x8 = np.ascontiguousarray(
                np.clip(xc[:, :K8], -240, 240)
                .reshape(M_TILES, P, Q8, 2, P)
                .transpose(0, 4, 2, 3, 1)).astype(E4)
            m["x8"] = x8
            m["w8"] = w8
        in_maps.append(m)
    return in_maps


def kernel(x, weight, bias, lora_A, lora_B):
    nc = _get_nc(1)
    in_maps = prep_in_maps(x, weight, bias, lora_A, lora_B)
    res = bass_utils.run_bass_kernel_spmd(nc, in_maps, core_ids=list(range(N_CORES)))
    out = np.concatenate([res.results[c]["out"] for c in range(N_CORES)], axis=0)
    return out.reshape(B, S, D_OUT)
